# revision 1
# baseline (speedup 1.0000x reference)
"""DeepSeekMoE Trainium2 kernel (8 NeuronCores, SPMD).

Strategy:
  - Host computes top-2 routing (sharding decision only) and gathers tokens
    into per-expert groups of capacity CAP, forming a "pairs" matrix
    [D, E*CAP].  Every core receives the same pairs; the routed experts are
    tensor-parallel over d_ff: core c computes f-slice [c*512, (c+1)*512) of
    every expert's FFN for every pair, producing a partial output that the
    host reduces over cores and scatter-adds back to token positions.
  - The gate (softmax over expert logits, prob of the block's expert) is
    computed on device: bf16 logits matmul into fp32 psum, Exp on ScalarE,
    partition reductions on GpSimd/VectorE (off the PE critical path),
    software-pipelined one expert ahead of the FFN that consumes it.
  - Shared experts are sharded 1:1: core c runs shared expert c over all
    T tokens; host sums the 8 contributions.  Shared chunks are interleaved
    between routed experts to keep the PE dense.
  - All matmuls bf16 (fp32 psum).  alpha/NS is folded into w2_s on the host
    (exact power-of-two scale); (1-alpha) is folded into the gate.
"""

import contextlib

import numpy as np
import ml_dtypes

import concourse.bacc as bacc
import concourse.bass_isa as bass_isa
import concourse.tile as tile
import concourse.mybir as mybir
from concourse.bass_utils import run_bass_kernel_spmd

BF16 = ml_dtypes.bfloat16

B, S, D, F, E, NS, K = 2, 1024, 1024, 4096, 8, 8, 2
T = B * S
FS = F // NS            # shared expert hidden = 512
FL = F // 8             # per-core f-slice of routed experts = 512
KCFG = {"cap": 576, "yo_act": True, "gate_first": False}
CAP = KCFG["cap"]       # per-expert token capacity (max observed 540)
NPAIR = E * CAP
ALPHA = 0.5
N_CORES = 8

_NC = None          # compiled program cache
LAST_RESULT = None  # BassKernelResults of the most recent run (for profiling)


def _build_program(cfg=None):
    cfg = cfg or {}
    cap = cfg.get("cap", CAP)
    yo_act = cfg.get("yo_act", True)       # psum->sbuf copy engine for yr
    gate_first = cfg.get("gate_first", False)  # gate(e+1) before ffn13(e)
    chunks = [(0, 512), (512, cap - 512)] if cap > 512 else [(0, cap)]

    bf = mybir.dt.bfloat16
    f32 = mybir.dt.float32
    Act = mybir.ActivationFunctionType
    Alu = mybir.AluOpType

    nc = bacc.Bacc("TRN2", target_bir_lowering=False, debug=False,
                   num_devices=N_CORES)

    npair = E * cap
    xp = nc.dram_tensor("xp", [D, npair], bf, kind="ExternalInput").ap()
    xf = nc.dram_tensor("xf", [D, T], bf, kind="ExternalInput").ap()
    gw = nc.dram_tensor("gw", [D, E], bf, kind="ExternalInput").ap()
    sel = nc.dram_tensor("sel", [E, E], f32, kind="ExternalInput").ap()
    w1l = nc.dram_tensor("w1l", [E, D, FL], bf, kind="ExternalInput").ap()
    w3l = nc.dram_tensor("w3l", [E, D, FL], bf, kind="ExternalInput").ap()
    w2l = nc.dram_tensor("w2l", [E, FL, D], bf, kind="ExternalInput").ap()
    w1s = nc.dram_tensor("w1s", [D, FS], bf, kind="ExternalInput").ap()
    w3s = nc.dram_tensor("w3s", [D, FS], bf, kind="ExternalInput").ap()
    w2s = nc.dram_tensor("w2s", [FS, D], bf, kind="ExternalInput").ap()
    yr = nc.dram_tensor("yr", [D, npair], bf, kind="ExternalOutput").ap()
    ys = nc.dram_tensor("ys", [D, T], bf, kind="ExternalOutput").ap()

    xp_r = xp.rearrange("(a p) t -> p a t", p=128)
    xf_r = xf.rearrange("(a p) t -> p a t", p=128)
    gw_r = gw.rearrange("(a p) e -> p a e", p=128)
    yr_r = yr.rearrange("(a p) t -> p a t", p=128)
    ys_r = ys.rearrange("(a p) t -> p a t", p=128)
    w1s_r = w1s.rearrange("(a p) f -> p a f", p=128)
    w3s_r = w3s.rearrange("(a p) f -> p a f", p=128)
    w2s_r = w2s.rearrange("(a p) d -> p a d", p=128)

    with tile.TileContext(nc) as tc:
        with contextlib.ExitStack() as ctx:
            const = ctx.enter_context(tc.tile_pool(name="const", bufs=1))
            wst = ctx.enter_context(tc.tile_pool(name="wst", bufs=2))
            acts = ctx.enter_context(tc.tile_pool(name="acts", bufs=3))
            hts = ctx.enter_context(tc.tile_pool(name="hts", bufs=2))
            gpool = ctx.enter_context(tc.tile_pool(name="gpool", bufs=2))
            outs = ctx.enter_context(tc.tile_pool(name="outs", bufs=2))
            psum = ctx.enter_context(
                tc.tile_pool(name="psum", bufs=3, space="PSUM"))
            psg = ctx.enter_context(
                tc.tile_pool(name="psg", bufs=2, space="PSUM"))

            state = {}

            def load_xp(e):
                XP = acts.tile([128, 8, cap], bf, tag="xp", name=f"xp{e}")
                nc.sync.dma_start(
                    out=XP, in_=xp_r[:, :, e * cap:(e + 1) * cap])
                state[("XP", e)] = XP

            def load_w(e, split=False):
                W1 = wst.tile([128, 8, FL], bf, tag="w1", name=f"w1_{e}")
                W3 = wst.tile([128, 8, FL], bf, tag="w3", name=f"w3_{e}")
                w1r = w1l[e].rearrange("(a p) f -> p a f", p=128)
                w3r = w3l[e].rearrange("(a p) f -> p a f", p=128)
                if split:  # f-subtile split so the first MMs can start early
                    for ft in range(4):
                        fsl = slice(ft * 128, (ft + 1) * 128)
                        nc.sync.dma_start(out=W1[:, :, fsl],
                                          in_=w1r[:, :, fsl])
                        nc.sync.dma_start(out=W3[:, :, fsl],
                                          in_=w3r[:, :, fsl])
                else:
                    nc.sync.dma_start(out=W1, in_=w1r)
                    nc.sync.dma_start(out=W3, in_=w3r)
                W2 = wst.tile([128, 4, D], bf, tag="w2", name=f"w2_{e}")
                nc.sync.dma_start(
                    out=W2, in_=w2l[e].rearrange("(a p) d -> p a d", p=128))
                state[("W", e)] = (W1, W3, W2)

            def load_xf(ch):
                o = ch * 512
                XF = acts.tile([128, 8, 512], bf, tag="xf", name=f"xf{ch}")
                nc.sync.dma_start(out=XF, in_=xf_r[:, :, o:o + 512])
                state[("XF", ch)] = XF

            def gate(e):
                """G[:, j] = (1-alpha) * softmax(logits[:, j])[e], bf16."""
                GW, SEL = state["GW"], state["SEL"]
                XP = state[("XP", e)]
                Ge = gpool.tile([128, cap], bf, tag="G", name=f"G{e}")
                DEN = gpool.tile([8, cap], f32, tag="den", name=f"dn{e}")
                NUM = gpool.tile([8, cap], f32, tag="num", name=f"nm{e}")
                for ci, (o, n) in enumerate(chunks):
                    lg = psg.tile([8, 512], f32, tag="lg", name=f"lg{e}_{ci}")
                    for dt in range(8):
                        nc.tensor.matmul(
                            lg[:, :n], GW[:, dt, :], XP[:, dt, o:o + n],
                            start=(dt == 0), stop=(dt == 7))
                    EXPt = gpool.tile([8, 512], f32, tag="exp",
                                      name=f"ex{e}_{ci}")
                    nc.scalar.activation(EXPt[:, :n], lg[:, :n], Act.Exp)
                    TMP = gpool.tile([8, 512], f32, tag="tmp",
                                     name=f"tm{e}_{ci}")
                    nc.vector.tensor_scalar_mul(TMP[:, :n], EXPt[:, :n],
                                                SEL[:, e:e + 1])
                    nc.gpsimd.partition_all_reduce(
                        DEN[:, o:o + n], EXPt[:, :n], channels=8,
                        reduce_op=bass_isa.ReduceOp.add)
                    nc.gpsimd.partition_all_reduce(
                        NUM[:, o:o + n], TMP[:, :n], channels=8,
                        reduce_op=bass_isa.ReduceOp.add)
                rden = gpool.tile([1, cap], f32, tag="rden", name=f"rd{e}")
                nc.vector.reciprocal(rden, DEN[0:1, :])
                grow = gpool.tile([1, cap], bf, tag="grow", name=f"gr{e}")
                nc.vector.scalar_tensor_tensor(
                    grow, NUM[0:1, :], 1.0 - ALPHA,
                    rden, Alu.mult, Alu.mult)
                nc.gpsimd.partition_broadcast(Ge, grow)
                state[("G", e)] = Ge

            def ffn13(e):
                XP = state[("XP", e)]
                W1, W3, _ = state[("W", e)]
                Ge = state[("G", e)]
                HT = hts.tile([128, 4, cap], bf, tag="ht", name=f"ht{e}")
                for ft in range(4):
                    p1 = psum.tile([128, 1024], f32, tag="ps",
                                   name=f"p1_{e}_{ft}")
                    p3 = psum.tile([128, 1024], f32, tag="ps",
                                   name=f"p3_{e}_{ft}")
                    for dt in range(8):
                        st, sp = dt == 0, dt == 7
                        lw1 = W1[:, dt, ft * 128:(ft + 1) * 128]
                        for (o, n) in chunks:
                            nc.tensor.matmul(p1[:, o:o + n], lw1,
                                             XP[:, dt, o:o + n],
                                             start=st, stop=sp)
                        lw3 = W3[:, dt, ft * 128:(ft + 1) * 128]
                        for (o, n) in chunks:
                            nc.tensor.matmul(p3[:, o:o + n], lw3,
                                             XP[:, dt, o:o + n],
                                             start=st, stop=sp)
                    for (o, n) in chunks:
                        sa = gpool.tile([128, 512], f32, tag="silu",
                                        name=f"sa{e}_{ft}_{o}")
                        nc.scalar.activation(sa[:, :n], p1[:, o:o + n],
                                             Act.Silu)
                        nc.vector.tensor_mul(HT[:, ft, o:o + n], sa[:, :n],
                                             p3[:, o:o + n])
                        nc.vector.tensor_mul(HT[:, ft, o:o + n],
                                             HT[:, ft, o:o + n],
                                             Ge[:, o:o + n])
                state[("HT", e)] = HT

            def mm2(e):
                W2 = state[("W", e)][2]
                HT = state[("HT", e)]
                yo = outs.tile([128, 8, cap], bf, tag="yo", name=f"yo{e}")
                for dt in range(8):
                    py = psum.tile([128, 1024], f32, tag="ps",
                                   name=f"py{e}_{dt}")
                    for ft in range(4):
                        st, sp = ft == 0, ft == 3
                        lw2 = W2[:, ft, dt * 128:(dt + 1) * 128]
                        for (o, n) in chunks:
                            nc.tensor.matmul(py[:, o:o + n], lw2,
                                             HT[:, ft, o:o + n],
                                             start=st, stop=sp)
                    if yo_act:
                        nc.scalar.activation(yo[:, dt, :], py[:, 0:cap],
                                             Act.Copy)
                    else:
                        nc.vector.tensor_copy(out=yo[:, dt, :],
                                              in_=py[:, 0:cap])
                nc.sync.dma_start(
                    out=yr_r[:, :, e * cap:(e + 1) * cap], in_=yo)

            def shared_chunk(ch):
                W1S, W3S, W2S = state["W1S"], state["W3S"], state["W2S"]
                o = ch * 512
                XF = state[("XF", ch)]
                HS = hts.tile([128, 4, 512], bf, tag="hs", name=f"hs{ch}")
                for ft in range(4):
                    p1 = psum.tile([128, 1024], f32, tag="ps",
                                   name=f"sp1_{ch}_{ft}")
                    p3 = psum.tile([128, 1024], f32, tag="ps",
                                   name=f"sp3_{ch}_{ft}")
                    for dt in range(8):
                        st, sp = dt == 0, dt == 7
                        nc.tensor.matmul(p1[:, 0:512],
                                         W1S[:, dt, ft * 128:(ft + 1) * 128],
                                         XF[:, dt, :], start=st, stop=sp)
                        nc.tensor.matmul(p3[:, 0:512],
                                         W3S[:, dt, ft * 128:(ft + 1) * 128],
                                         XF[:, dt, :], start=st, stop=sp)
                    sa = gpool.tile([128, 512], f32, tag="silu",
                                    name=f"ssa{ch}_{ft}")
                    nc.scalar.activation(sa, p1[:, 0:512], Act.Silu)
                    nc.vector.tensor_mul(HS[:, ft, :], sa, p3[:, 0:512])
                so = outs.tile([128, 8, 512], bf, tag="so", name=f"so{ch}")
                for dt in range(8):
                    py = psum.tile([128, 1024], f32, tag="ps",
                                   name=f"spy{ch}_{dt}")
                    for ft in range(4):
                        nc.tensor.matmul(py[:, 0:512],
                                         W2S[:, ft, dt * 128:(dt + 1) * 128],
                                         HS[:, ft, :],
                                         start=(ft == 0), stop=(ft == 3))
                    nc.scalar.activation(so[:, dt, :], py[:, 0:512], Act.Copy)
                nc.sync.dma_start(out=ys_r[:, :, o:o + 512], in_=so)

            # ---- prologue: DMAs in consumption order -------------------
            load_xf(0)
            W1S = const.tile([128, 8, FS], bf)
            nc.sync.dma_start(out=W1S, in_=w1s_r)
            W3S = const.tile([128, 8, FS], bf)
            nc.sync.dma_start(out=W3S, in_=w3s_r)
            W2S = const.tile([128, 4, D], bf)
            nc.sync.dma_start(out=W2S, in_=w2s_r)
            GW = const.tile([128, 8, E], bf)
            nc.sync.dma_start(out=GW, in_=gw_r)
            SEL = const.tile([E, E], f32)
            nc.sync.dma_start(out=SEL, in_=sel)
            state.update(W1S=W1S, W3S=W3S, W2S=W2S, GW=GW, SEL=SEL)
            load_xp(0)
            load_w(0, split=True)

            shared_chunk(0)      # fills the PE while expert-0 inputs stream
            gate(0)
            load_xp(1)
            load_w(1)
            load_xf(1)
            for e in range(E):
                if e + 2 < E:
                    load_xp(e + 2)
                if e + 1 < E:
                    load_w(e + 1)
                if e in (0, 2):
                    load_xf(e // 2 + 2)
                if gate_first and e + 1 < E:
                    gate(e + 1)
                ffn13(e)
                if (not gate_first) and e + 1 < E:
                    gate(e + 1)
                mm2(e)
                if e in (1, 3, 5):
                    shared_chunk((e + 1) // 2)

    nc.compile()
    return nc


def _get_program():
    global _NC
    if _NC is None:
        _NC = _build_program(KCFG)
    return _NC


def kernel(hidden_states, gate_W, w1_e, w3_e, w2_e, w1_s, w3_s, w2_s):
    global LAST_RESULT
    x = np.ascontiguousarray(np.asarray(hidden_states, np.float32).reshape(T, D))

    # ---- host routing (sharding decision) ---------------------------
    gate_W = np.asarray(gate_W, np.float32)
    logits = x @ gate_W.T                       # [T, E]
    m = logits.max(axis=1, keepdims=True)
    p = np.exp(logits - m)
    probs = p / p.sum(axis=1, keepdims=True)
    order = np.argsort(-probs, axis=1, kind="stable")[:, :K]   # [T, K]

    idx = []            # token indices routed to each expert
    for e in range(E):
        te = np.where((order == e).any(axis=1))[0]
        if len(te) > CAP:   # graceful over-capacity: keep highest-prob tokens
            keep = np.argsort(-probs[te, e], kind="stable")[:CAP]
            te = np.sort(te[keep])
        idx.append(te)

    # ---- build device inputs ----------------------------------------
    xT = np.ascontiguousarray(x.T)              # [D, T] fp32
    xf_bf = xT.astype(BF16)                     # [D, T]
    xp_bf = np.zeros((D, NPAIR), dtype=BF16)
    for e in range(E):
        te = idx[e]
        xp_bf[:, e * CAP: e * CAP + len(te)] = xf_bf[:, te]

    gw_bf = np.ascontiguousarray(gate_W.T).astype(BF16)      # [D, E]
    w1_e = np.asarray(w1_e, np.float32)
    w3_e = np.asarray(w3_e, np.float32)
    w2_e = np.asarray(w2_e, np.float32)
    w1_s = np.asarray(w1_s, np.float32)
    w3_s = np.asarray(w3_s, np.float32)
    # fold alpha/NS (an exact power of two) into the shared down-proj
    w2_s = np.asarray(w2_s, np.float32) * (ALPHA / NS)

    nc = _get_program()
    in_maps = []
    for c in range(N_CORES):
        fsl = slice(c * FL, (c + 1) * FL)
        in_maps.append({
            "xp": xp_bf,
            "xf": xf_bf,
            "gw": gw_bf,
            "sel": np.eye(E, dtype=np.float32),
            "w1l": np.ascontiguousarray(w1_e[:, :, fsl]).astype(BF16),
            "w3l": np.ascontiguousarray(w3_e[:, :, fsl]).astype(BF16),
            "w2l": np.ascontiguousarray(w2_e[:, fsl, :]).astype(BF16),
            "w1s": w1_s[c].astype(BF16),
            "w3s": w3_s[c].astype(BF16),
            "w2s": w2_s[c].astype(BF16),
        })

    res = run_bass_kernel_spmd(nc, in_maps, list(range(N_CORES)))
    LAST_RESULT = res

    # ---- host combine (unshard) -------------------------------------
    outT = np.zeros((D, T), np.float32)
    yr_sum = np.zeros((D, NPAIR), np.float32)
    for c in range(N_CORES):
        yr_sum += res.results[c]["yr"].astype(np.float32)
        outT += res.results[c]["ys"].astype(np.float32)
    for e in range(E):
        te = idx[e]
        outT[:, te] += yr_sum[:, e * CAP: e * CAP + len(te)]

    return np.ascontiguousarray(outT.T).reshape(B, S, D).astype(np.float32)



# revision 2
# speedup vs baseline: 1.2107x; 1.2107x over previous
"""DeepSeekMoE Trainium2 kernel (8 NeuronCores, SPMD).

Strategy (v2 — uniform item loop, no on-device gate):
  - Host computes top-2 routing and packs the routed tokens into exact-size
    per-expert groups (sum = T*K = 4096 columns, no capacity padding),
    then appends all T tokens once more as the "shared" group (2048 cols):
    one pairs matrix xp [D, 6144] = exactly 12 items of 512 columns.
  - The 8 shared experts (hidden FS=512 each, averaged) are algebraically
    one big FFN with hidden 8*512=4096; its per-core d_ff slice is exactly
    shared expert c.  So every core sees 9 uniform weight sets
    ([1024,512]/[1024,512]/[512,1024]): routed experts 0..7 (f-slice c)
    + shared expert c as weight-set 8.  alpha/NS (=1/16, exact) is folded
    into the shared w2 on the host.
  - Device: per item, ffn13 (w1/w3, 8 dt x 4 ft accumulation) -> silu*mul
    -> mm2 (w2) -> psum->sbuf bf16 copy -> DMA out.  Items may span expert
    boundaries; the per-(dt,ft) matmul is split at group boundaries
    (few ns extra per split).  mm2 is software-pipelined one item behind
    ffn13 so the silu/mul latency of the last ft never stalls the PE.
  - No gating on device: host multiplies routed outputs by (1-alpha)*prob
    during the combine (the standard MoE weighted-combine), so the PE does
    nothing but the 12*96 N=512 matmuls = the bf16 roofline.
  - Inputs stream on the Sync HWDGE queue in exact consumption order;
    outputs go on the Scalar HWDGE queue so their compute-gated waits can
    never head-of-line-block the input supply.
"""

import contextlib

import numpy as np
import ml_dtypes

import concourse.bacc as bacc
import concourse.tile as tile
import concourse.mybir as mybir
from concourse.bass_utils import run_bass_kernel_spmd

BF16 = ml_dtypes.bfloat16

B, S, D, F, E, NS, K = 2, 1024, 1024, 4096, 8, 8, 2
T = B * S
FS = F // NS            # shared expert hidden = 512 (= per-core routed slice)
FL = F // 8             # per-core f-slice of routed experts = 512
NW = E + 1              # 9 weight sets; wid 8 = shared
NCOL = T * K + T        # 6144 pair columns
IW = 512                # item width
NIT = NCOL // IW        # 12 items
ALPHA = 0.5
N_CORES = 8

# PE order of the 12 items: routed items r0..r7 occupy cols [512j, 512j+512),
# shared items s0..s3 are items 8..11 (cols 4096+).  Two shared items lead
# (tiny DMA footprint -> cheap prologue), the rest interleave for DMA slack.
PE_ORDER = [8, 9, 0, 1, 10, 2, 3, 11, 4, 5, 6, 7]

_CACHE = {}         # sizes tuple -> compiled program
LAST_RESULT = None  # BassKernelResults of the most recent run (for profiling)


def _items_from_sizes(sizes):
    """Per-item segment lists [(wid, col_off_in_item, len), ...]."""
    bounds = np.cumsum([0] + list(sizes))
    items = []
    for j in range(8):                       # routed items
        lo, hi = IW * j, IW * j + IW
        segs = []
        for e in range(E):
            a, b = max(lo, int(bounds[e])), min(hi, int(bounds[e + 1]))
            if b > a:
                segs.append((e, a - lo, b - a))
        items.append(segs)
    for c in range(4):                       # shared items
        items.append([(E, 0, IW)])
    return items


def _build_program(sizes):
    bf = mybir.dt.bfloat16
    f32 = mybir.dt.float32
    Act = mybir.ActivationFunctionType

    items = _items_from_sizes(sizes)
    # first PE position at which each routed wid is used
    first_use = {}
    for pos, it in enumerate(PE_ORDER):
        for (wid, _, _) in items[it]:
            if wid != E and wid not in first_use:
                first_use[wid] = pos
    # weight-issue schedule: wid issued at block (first_use - 2)
    w_sched = {}
    for wid, fu in sorted(first_use.items(), key=lambda kv: kv[1]):
        w_sched.setdefault(max(0, fu - 2), []).append(wid)

    nc = bacc.Bacc("TRN2", target_bir_lowering=False, debug=False,
                   num_devices=N_CORES)

    xp = nc.dram_tensor("xp", [D, NCOL], bf, kind="ExternalInput").ap()
    w1 = nc.dram_tensor("w1", [NW, D, FL], bf, kind="ExternalInput").ap()
    w3 = nc.dram_tensor("w3", [NW, D, FL], bf, kind="ExternalInput").ap()
    w2 = nc.dram_tensor("w2", [NW, FL, D], bf, kind="ExternalInput").ap()
    y = nc.dram_tensor("y", [NIT, 128, 8, IW], bf, kind="ExternalOutput").ap()

    xp_r = xp.rearrange("(a p) t -> p a t", p=128)
    w1_r = [w1[i].rearrange("(a p) f -> p a f", p=128) for i in range(NW)]
    w3_r = [w3[i].rearrange("(a p) f -> p a f", p=128) for i in range(NW)]
    w2_r = [w2[i].rearrange("(a p) d -> p a d", p=128) for i in range(NW)]

    with tile.TileContext(nc) as tc:
        with contextlib.ExitStack() as ctx:
            const = ctx.enter_context(tc.tile_pool(name="const", bufs=1))
            wst = ctx.enter_context(tc.tile_pool(name="wst", bufs=4))
            acts = ctx.enter_context(tc.tile_pool(name="acts", bufs=4))
            hts = ctx.enter_context(tc.tile_pool(name="hts", bufs=2))
            spool = ctx.enter_context(tc.tile_pool(name="spool", bufs=2))
            outs = ctx.enter_context(tc.tile_pool(name="outs", bufs=2))
            psum = ctx.enter_context(
                tc.tile_pool(name="psum", bufs=2, space="PSUM"))

            state = {}

            def load_w(wid):
                if wid == E:   # shared set: const pool, ft-split w1/w3
                    W1 = const.tile([128, 8, FL], bf, name="w1s")
                    W3 = const.tile([128, 8, FL], bf, name="w3s")
                    W2 = const.tile([128, 4, D], bf, name="w2s")
                else:
                    W1 = wst.tile([128, 8, FL], bf, tag="w1", name=f"w1_{wid}")
                    W3 = wst.tile([128, 8, FL], bf, tag="w3", name=f"w3_{wid}")
                    W2 = wst.tile([128, 4, D], bf, tag="w2", name=f"w2_{wid}")
                    nc.sync.dma_start(out=W1, in_=w1_r[wid])
                    nc.sync.dma_start(out=W3, in_=w3_r[wid])
                    nc.sync.dma_start(out=W2, in_=w2_r[wid])
                state[("W", wid)] = (W1, W3, W2)
                return W1, W3, W2

            def load_xp(pos):
                it = PE_ORDER[pos]
                o = it * IW
                XP = acts.tile([128, 8, IW], bf, tag="xp", name=f"xp{it}")
                nc.sync.dma_start(out=XP, in_=xp_r[:, :, o:o + IW])
                state[("XP", it)] = XP

            def ffn13(it):
                XP = state[("XP", it)]
                HT = hts.tile([128, 4, IW], bf, tag="ht", name=f"ht{it}")
                for ft in range(4):
                    fsl = slice(ft * 128, (ft + 1) * 128)
                    p1 = psum.tile([128, IW], f32, tag="p1", name=f"p1_{it}_{ft}")
                    p3 = psum.tile([128, IW], f32, tag="p3", name=f"p3_{it}_{ft}")
                    for wi, ps in ((0, p1), (1, p3)):
                        for (wid, o, ln) in items[it]:
                            W = state[("W", wid)][wi]
                            for dt in range(8):
                                nc.tensor.matmul(
                                    ps[:, o:o + ln], W[:, dt, fsl],
                                    XP[:, dt, o:o + ln],
                                    start=(dt == 0), stop=(dt == 7))
                    sa = spool.tile([128, IW], f32, tag="sa", name=f"sa{it}_{ft}")
                    nc.scalar.activation(sa, p1, Act.Silu)
                    nc.vector.tensor_mul(HT[:, ft, :], sa, p3)
                state[("HT", it)] = HT

            def mm2(it, stream_out=False):
                HT = state[("HT", it)]
                yo = outs.tile([128, 8, IW], bf, tag="yo", name=f"yo{it}")
                for dt in range(8):
                    dsl = slice(dt * 128, (dt + 1) * 128)
                    py = psum.tile([128, IW], f32, tag="py", name=f"py{it}_{dt}")
                    for (wid, o, ln) in items[it]:
                        W2t = state[("W", wid)][2]
                        for ft in range(4):
                            nc.tensor.matmul(
                                py[:, o:o + ln], W2t[:, ft, dsl],
                                HT[:, ft, o:o + ln],
                                start=(ft == 0), stop=(ft == 3))
                    nc.vector.tensor_copy(out=yo[:, dt, :], in_=py)
                    if stream_out:
                        nc.scalar.dma_start(out=y[it, :, dt, :],
                                            in_=yo[:, dt, :])
                if not stream_out:
                    nc.scalar.dma_start(out=y[it], in_=yo)

            # ---- prologue: DMAs in exact consumption order -------------
            it0 = PE_ORDER[0]
            W1S, W3S, W2S = load_w(E)
            XP0 = acts.tile([128, 8, IW], bf, tag="xp", name=f"xp{it0}")
            state[("XP", it0)] = XP0
            o0 = it0 * IW
            nc.sync.dma_start(out=W1S[:, :, 0:128], in_=w1_r[E][:, :, 0:128])
            nc.sync.dma_start(out=XP0[:, 0:2, :], in_=xp_r[:, 0:2, o0:o0 + IW])
            nc.sync.dma_start(out=W3S[:, :, 0:128], in_=w3_r[E][:, :, 0:128])
            for q in range(1, 4):
                nc.sync.dma_start(out=XP0[:, 2 * q:2 * q + 2, :],
                                  in_=xp_r[:, 2 * q:2 * q + 2, o0:o0 + IW])
            for ft in range(1, 4):
                fsl = slice(ft * 128, (ft + 1) * 128)
                nc.sync.dma_start(out=W1S[:, :, fsl], in_=w1_r[E][:, :, fsl])
                nc.sync.dma_start(out=W3S[:, :, fsl], in_=w3_r[E][:, :, fsl])
            nc.sync.dma_start(out=W2S, in_=w2_r[E])
            load_xp(1)
            for wid in w_sched.get(0, []):
                load_w(wid)
            load_xp(2)

            # ---- main loop: mm2 lags ffn13 by one item -----------------
            for pos in range(NIT):
                if pos >= 1:
                    for wid in w_sched.get(pos, []):
                        load_w(wid)
                    if pos + 2 < NIT:
                        load_xp(pos + 2)
                ffn13(PE_ORDER[pos])
                if pos >= 1:
                    mm2(PE_ORDER[pos - 1])
            mm2(PE_ORDER[NIT - 1], stream_out=True)

    nc.compile()
    return nc


def kernel(hidden_states, gate_W, w1_e, w3_e, w2_e, w1_s, w3_s, w2_s):
    global LAST_RESULT
    x = np.ascontiguousarray(np.asarray(hidden_states, np.float32).reshape(T, D))

    # ---- host routing (sharding decision) + combine coefficients ----
    gate_W = np.asarray(gate_W, np.float32)
    logits = x @ gate_W.T                       # [T, E]
    m = logits.max(axis=1, keepdims=True)
    p = np.exp(logits - m)
    probs = p / p.sum(axis=1, keepdims=True)
    order = np.argsort(-probs, axis=1, kind="stable")[:, :K]   # [T, K]

    idx = [np.where((order == e).any(axis=1))[0] for e in range(E)]
    sizes = tuple(len(te) for te in idx)
    assert sum(sizes) == T * K

    # ---- build device inputs ----------------------------------------
    xT = np.ascontiguousarray(x.T)              # [D, T] fp32
    xf_bf = xT.astype(BF16)                     # [D, T]
    xp_bf = np.empty((D, NCOL), dtype=BF16)
    off = 0
    for e in range(E):
        n = sizes[e]
        xp_bf[:, off:off + n] = xf_bf[:, idx[e]]
        off += n
    xp_bf[:, T * K:] = xf_bf                    # shared group: all tokens

    w1_e = np.asarray(w1_e, np.float32)
    w3_e = np.asarray(w3_e, np.float32)
    w2_e = np.asarray(w2_e, np.float32)
    w1_s = np.asarray(w1_s, np.float32)
    w3_s = np.asarray(w3_s, np.float32)
    # fold alpha/NS (an exact power of two) into the shared down-proj
    w2_s = np.asarray(w2_s, np.float32) * (ALPHA / NS)

    nc = _CACHE.get(sizes)
    if nc is None:
        nc = _CACHE[sizes] = _build_program(sizes)

    in_maps = []
    for c in range(N_CORES):
        fsl = slice(c * FL, (c + 1) * FL)
        w1c = np.concatenate(
            [np.ascontiguousarray(w1_e[:, :, fsl]), w1_s[c:c + 1]], axis=0)
        w3c = np.concatenate(
            [np.ascontiguousarray(w3_e[:, :, fsl]), w3_s[c:c + 1]], axis=0)
        w2c = np.concatenate(
            [np.ascontiguousarray(w2_e[:, fsl, :]), w2_s[c:c + 1]], axis=0)
        in_maps.append({
            "xp": xp_bf,
            "w1": w1c.astype(BF16),
            "w3": w3c.astype(BF16),
            "w2": w2c.astype(BF16),
        })

    res = run_bass_kernel_spmd(nc, in_maps, list(range(N_CORES)))
    LAST_RESULT = res

    # ---- host combine (unshard + weighted MoE combine) --------------
    yfull = np.zeros((NIT, 128, 8, IW), np.float32)
    for c in range(N_CORES):
        yfull += res.results[c]["y"].astype(np.float32)
    # [it, p, a, t] -> [a*128+p, it*512+t] = [D, NCOL]
    yfull = np.ascontiguousarray(yfull.transpose(2, 1, 0, 3)).reshape(D, NCOL)

    outT = yfull[:, T * K:].copy()              # shared part (scales folded)
    off = 0
    for e in range(E):
        te = idx[e]
        coef = ((1.0 - ALPHA) * probs[te, e]).astype(np.float32)
        outT[:, te] += yfull[:, off:off + len(te)] * coef[None, :]
        off += len(te)

    return np.ascontiguousarray(outT.T).reshape(B, S, D).astype(np.float32)


# revision 9
# speedup vs baseline: 1.2228x; 1.0100x over previous
"""DeepSeekMoE Trainium2 kernel (8 NeuronCores, SPMD).

Strategy (v2 — uniform item loop, no on-device gate):
  - Host computes top-2 routing and packs the routed tokens into exact-size
    per-expert groups (sum = T*K = 4096 columns, no capacity padding),
    then appends all T tokens once more as the "shared" group (2048 cols):
    one pairs matrix xp [D, 6144] = exactly 12 items of 512 columns.
  - The 8 shared experts (hidden FS=512 each, averaged) are algebraically
    one big FFN with hidden 8*512=4096; its per-core d_ff slice is exactly
    shared expert c.  So every core sees 9 uniform weight sets
    ([1024,512]/[1024,512]/[512,1024]): routed experts 0..7 (f-slice c)
    + shared expert c as weight-set 8.  alpha/NS (=1/16, exact) is folded
    into the shared w2 on the host.
  - Device: per item, ffn13 (w1/w3, 8 dt x 4 ft accumulation) -> silu*mul
    -> mm2 (w2) -> psum->sbuf bf16 copy -> DMA out.  Items may span expert
    boundaries; the per-(dt,ft) matmul is split at group boundaries
    (few ns extra per split).  mm2 is software-pipelined one item behind
    ffn13 so the silu/mul latency of the last ft never stalls the PE.
  - No gating on device: host multiplies routed outputs by (1-alpha)*prob
    during the combine (the standard MoE weighted-combine), so the PE does
    nothing but the 12*96 N=512 matmuls = the bf16 roofline.
  - Inputs stream on the Sync HWDGE queue in exact consumption order;
    outputs go on the Scalar HWDGE queue so their compute-gated waits can
    never head-of-line-block the input supply.
"""

import contextlib

import numpy as np
import ml_dtypes

import concourse.bacc as bacc
import concourse.tile as tile
import concourse.mybir as mybir
from concourse.bass_utils import run_bass_kernel_spmd

BF16 = ml_dtypes.bfloat16

B, S, D, F, E, NS, K = 2, 1024, 1024, 4096, 8, 8, 2
T = B * S
FS = F // NS            # shared expert hidden = 512 (= per-core routed slice)
FL = F // 8             # per-core f-slice of routed experts = 512
NW = E + 1              # 9 weight sets; wid 8 = shared
NCOL = T * K + T        # 6144 pair columns
IW = 512                # item width
NIT = NCOL // IW        # 12 items
ALPHA = 0.5
N_CORES = 8

# PE order of the 12 items: routed items r0..r7 occupy cols [512j, 512j+512),
# shared items s0..s3 are items 8..11 (cols 4096+).  Two shared items lead
# (tiny DMA footprint -> cheap prologue), the rest interleave for DMA slack.
PE_ORDER = [8, 9, 0, 1, 10, 2, 3, 11, 4, 5, 6, 7]

KCFG = {
    "warmup_mms": 32,    # garbage matmuls to warm the HAM clock gate
    "py_bufs": 3,
    "stream_last": 2,    # per-dt output DMA for the last N items
    "out_queue": "sync", # engine queue for output DMAs
}

_CACHE = {}         # sizes tuple -> compiled program
LAST_RESULT = None  # BassKernelResults of the most recent run (for profiling)


def _items_from_sizes(sizes):
    """Per-item segment lists [(wid, col_off_in_item, len), ...]."""
    bounds = np.cumsum([0] + list(sizes))
    items = []
    for j in range(8):                       # routed items
        lo, hi = IW * j, IW * j + IW
        segs = []
        for e in range(E):
            a, b = max(lo, int(bounds[e])), min(hi, int(bounds[e + 1]))
            if b > a:
                segs.append((e, a - lo, b - a))
        items.append(segs)
    for c in range(4):                       # shared items
        items.append([(E, 0, IW)])
    return items


def _build_program(sizes):
    bf = mybir.dt.bfloat16
    f32 = mybir.dt.float32
    Act = mybir.ActivationFunctionType

    items = _items_from_sizes(sizes)
    # first PE position at which each routed wid is used
    first_use = {}
    for pos, it in enumerate(PE_ORDER):
        for (wid, _, _) in items[it]:
            if wid != E and wid not in first_use:
                first_use[wid] = pos
    # weight-issue schedule: wid issued at block (first_use - 2)
    w_sched = {}
    for wid, fu in sorted(first_use.items(), key=lambda kv: kv[1]):
        w_sched.setdefault(max(0, fu - 2), []).append(wid)

    nc = bacc.Bacc("TRN2", target_bir_lowering=False, debug=False,
                   num_devices=N_CORES)

    xp = nc.dram_tensor("xp", [D, NCOL], bf, kind="ExternalInput").ap()
    w1 = nc.dram_tensor("w1", [NW, D, FL], bf, kind="ExternalInput").ap()
    w3 = nc.dram_tensor("w3", [NW, D, FL], bf, kind="ExternalInput").ap()
    w2 = nc.dram_tensor("w2", [NW, FL, D], bf, kind="ExternalInput").ap()
    y = nc.dram_tensor("y", [NIT, 128, 8, IW], bf, kind="ExternalOutput").ap()

    xp_r = xp.rearrange("(a p) t -> p a t", p=128)
    w1_r = [w1[i].rearrange("(a p) f -> p a f", p=128) for i in range(NW)]
    w3_r = [w3[i].rearrange("(a p) f -> p a f", p=128) for i in range(NW)]
    w2_r = [w2[i].rearrange("(a p) d -> p a d", p=128) for i in range(NW)]

    with tile.TileContext(nc) as tc:
        with contextlib.ExitStack() as ctx:
            const = ctx.enter_context(tc.tile_pool(name="const", bufs=1))
            wst = ctx.enter_context(tc.tile_pool(name="wst", bufs=4))
            acts = ctx.enter_context(tc.tile_pool(name="acts", bufs=4))
            hts = ctx.enter_context(tc.tile_pool(name="hts", bufs=2))
            spool = ctx.enter_context(tc.tile_pool(name="spool", bufs=2))
            outs = ctx.enter_context(tc.tile_pool(name="outs", bufs=2))
            psum = ctx.enter_context(
                tc.tile_pool(name="psum", bufs=2, space="PSUM"))
            psy = ctx.enter_context(
                tc.tile_pool(name="psy", bufs=KCFG["py_bufs"], space="PSUM"))
            psw = ctx.enter_context(
                tc.tile_pool(name="psw", bufs=1, space="PSUM"))

            state = {}
            out_dma = nc.sync.dma_start if KCFG["out_queue"] == "sync" \
                else nc.scalar.dma_start

            def load_w(wid):
                if wid == E:   # shared set: const pool, ft-split w1/w3
                    W1 = const.tile([128, 8, FL], bf, name="w1s")
                    W3 = const.tile([128, 8, FL], bf, name="w3s")
                    W2 = const.tile([128, 4, D], bf, name="w2s")
                else:
                    W1 = wst.tile([128, 8, FL], bf, tag="w1", name=f"w1_{wid}")
                    W3 = wst.tile([128, 8, FL], bf, tag="w3", name=f"w3_{wid}")
                    W2 = wst.tile([128, 4, D], bf, tag="w2", name=f"w2_{wid}")
                    nc.sync.dma_start(out=W1, in_=w1_r[wid])
                    nc.sync.dma_start(out=W3, in_=w3_r[wid])
                    nc.sync.dma_start(out=W2, in_=w2_r[wid])
                state[("W", wid)] = (W1, W3, W2)
                return W1, W3, W2

            def load_xp(pos):
                it = PE_ORDER[pos]
                o = it * IW
                XP = acts.tile([128, 8, IW], bf, tag="xp", name=f"xp{it}")
                nc.sync.dma_start(out=XP, in_=xp_r[:, :, o:o + IW])
                state[("XP", it)] = XP

            def ffn13(it):
                XP = state[("XP", it)]
                HT = hts.tile([128, 4, IW], bf, tag="ht", name=f"ht{it}")
                for ft in range(4):
                    fsl = slice(ft * 128, (ft + 1) * 128)
                    p1 = psum.tile([128, IW], f32, tag="p1", name=f"p1_{it}_{ft}")
                    p3 = psum.tile([128, IW], f32, tag="p3", name=f"p3_{it}_{ft}")
                    for wi, ps in ((0, p1), (1, p3)):
                        for (wid, o, ln) in items[it]:
                            W = state[("W", wid)][wi]
                            for dt in range(8):
                                nc.tensor.matmul(
                                    ps[:, o:o + ln], W[:, dt, fsl],
                                    XP[:, dt, o:o + ln],
                                    start=(dt == 0), stop=(dt == 7))
                    sa = spool.tile([128, IW], f32, tag="sa", name=f"sa{it}_{ft}")
                    nc.scalar.activation(sa, p1, Act.Silu)
                    nc.vector.tensor_mul(HT[:, ft, :], sa, p3)
                state[("HT", it)] = HT

            def mm2(it, stream_out=False):
                HT = state[("HT", it)]
                yo = outs.tile([128, 8, IW], bf, tag="yo", name=f"yo{it}")
                for dt in range(8):
                    dsl = slice(dt * 128, (dt + 1) * 128)
                    py = psy.tile([128, IW], f32, tag="py", name=f"py{it}_{dt}")
                    for (wid, o, ln) in items[it]:
                        W2t = state[("W", wid)][2]
                        for ft in range(4):
                            nc.tensor.matmul(
                                py[:, o:o + ln], W2t[:, ft, dsl],
                                HT[:, ft, o:o + ln],
                                start=(ft == 0), stop=(ft == 3))
                    nc.vector.tensor_copy(out=yo[:, dt, :], in_=py)
                    if stream_out:
                        out_dma(out=y[it, :, dt, :], in_=yo[:, dt, :])
                if not stream_out:
                    out_dma(out=y[it], in_=yo)

            # ---- HAM warmup: garbage matmuls fill the DMA-dead window --
            # The PE clock gate (HAM) needs ~3.4us of sustained activity to
            # un-throttle from 1.2 to 2.4 GHz.  Data DMAs cannot land before
            # ~9us, so burn that window on matmuls over uninitialized SBUF;
            # by the time real matmuls issue, the PE is already warm.
            nwu = KCFG["warmup_mms"]
            if nwu:
                wub = const.tile([128, 128], bf, name="wub")
                wup = psw.tile([128, 128], f32, tag="wu", name="wup")
                nc.vector.memset(wub, 0.0)
                for i in range(nwu):
                    nc.tensor.matmul(wup, wub, wub, start=True, stop=True)

            # ---- prologue: DMAs in exact consumption order -------------
            it0 = PE_ORDER[0]
            W1S, W3S, W2S = load_w(E)
            XP0 = acts.tile([128, 8, IW], bf, tag="xp", name=f"xp{it0}")
            state[("XP", it0)] = XP0
            o0 = it0 * IW
            nc.sync.dma_start(out=W1S[:, :, 0:128], in_=w1_r[E][:, :, 0:128])
            for q in range(4):
                nc.sync.dma_start(out=XP0[:, 2 * q:2 * q + 2, :],
                                  in_=xp_r[:, 2 * q:2 * q + 2, o0:o0 + IW])
                if q == 1:
                    nc.sync.dma_start(out=W3S[:, :, 0:128],
                                      in_=w3_r[E][:, :, 0:128])
            for ft in range(1, 4):
                fsl = slice(ft * 128, (ft + 1) * 128)
                nc.sync.dma_start(out=W1S[:, :, fsl], in_=w1_r[E][:, :, fsl])
                nc.sync.dma_start(out=W3S[:, :, fsl], in_=w3_r[E][:, :, fsl])
            nc.sync.dma_start(out=W2S, in_=w2_r[E])
            load_xp(1)
            for wid in w_sched.get(0, []):
                load_w(wid)
            load_xp(2)

            # ---- main loop: mm2 lags ffn13 by one item -----------------
            nstream = KCFG["stream_last"]
            for pos in range(NIT):
                if pos >= 1:
                    for wid in w_sched.get(pos, []):
                        load_w(wid)
                    if pos + 2 < NIT:
                        load_xp(pos + 2)
                ffn13(PE_ORDER[pos])
                if pos >= 1:
                    mm2(PE_ORDER[pos - 1],
                        stream_out=(pos - 1 >= NIT - nstream))
            mm2(PE_ORDER[NIT - 1], stream_out=True)

    nc.compile()
    return nc


def kernel(hidden_states, gate_W, w1_e, w3_e, w2_e, w1_s, w3_s, w2_s):
    global LAST_RESULT
    x = np.ascontiguousarray(np.asarray(hidden_states, np.float32).reshape(T, D))

    # ---- host routing (sharding decision) + combine coefficients ----
    gate_W = np.asarray(gate_W, np.float32)
    logits = x @ gate_W.T                       # [T, E]
    m = logits.max(axis=1, keepdims=True)
    p = np.exp(logits - m)
    probs = p / p.sum(axis=1, keepdims=True)
    order = np.argsort(-probs, axis=1, kind="stable")[:, :K]   # [T, K]

    idx = [np.where((order == e).any(axis=1))[0] for e in range(E)]
    sizes = tuple(len(te) for te in idx)
    assert sum(sizes) == T * K

    # ---- build device inputs ----------------------------------------
    xT = np.ascontiguousarray(x.T)              # [D, T] fp32
    xf_bf = xT.astype(BF16)                     # [D, T]
    xp_bf = np.empty((D, NCOL), dtype=BF16)
    off = 0
    for e in range(E):
        n = sizes[e]
        xp_bf[:, off:off + n] = xf_bf[:, idx[e]]
        off += n
    xp_bf[:, T * K:] = xf_bf                    # shared group: all tokens

    w1_e = np.asarray(w1_e, np.float32)
    w3_e = np.asarray(w3_e, np.float32)
    w2_e = np.asarray(w2_e, np.float32)
    w1_s = np.asarray(w1_s, np.float32)
    w3_s = np.asarray(w3_s, np.float32)
    # fold alpha/NS (an exact power of two) into the shared down-proj
    w2_s = np.asarray(w2_s, np.float32) * (ALPHA / NS)

    nc = _CACHE.get(sizes)
    if nc is None:
        nc = _CACHE[sizes] = _build_program(sizes)

    in_maps = []
    for c in range(N_CORES):
        fsl = slice(c * FL, (c + 1) * FL)
        w1c = np.concatenate(
            [np.ascontiguousarray(w1_e[:, :, fsl]), w1_s[c:c + 1]], axis=0)
        w3c = np.concatenate(
            [np.ascontiguousarray(w3_e[:, :, fsl]), w3_s[c:c + 1]], axis=0)
        w2c = np.concatenate(
            [np.ascontiguousarray(w2_e[:, fsl, :]), w2_s[c:c + 1]], axis=0)
        in_maps.append({
            "xp": xp_bf,
            "w1": w1c.astype(BF16),
            "w3": w3c.astype(BF16),
            "w2": w2c.astype(BF16),
        })

    res = run_bass_kernel_spmd(nc, in_maps, list(range(N_CORES)))
    LAST_RESULT = res

    # ---- host combine (unshard + weighted MoE combine) --------------
    yfull = np.zeros((NIT, 128, 8, IW), np.float32)
    for c in range(N_CORES):
        yfull += res.results[c]["y"].astype(np.float32)
    # [it, p, a, t] -> [a*128+p, it*512+t] = [D, NCOL]
    yfull = np.ascontiguousarray(yfull.transpose(2, 1, 0, 3)).reshape(D, NCOL)

    outT = yfull[:, T * K:].copy()              # shared part (scales folded)
    off = 0
    for e in range(E):
        te = idx[e]
        coef = ((1.0 - ALPHA) * probs[te, e]).astype(np.float32)
        outT[:, te] += yfull[:, off:off + len(te)] * coef[None, :]
        off += len(te)

    return np.ascontiguousarray(outT.T).reshape(B, S, D).astype(np.float32)


# revision 15
# speedup vs baseline: 1.2393x; 1.0136x over previous
"""DeepSeekMoE Trainium2 kernel (8 NeuronCores, SPMD).

Strategy (v2 — uniform item loop, no on-device gate):
  - Host computes top-2 routing and packs the routed tokens into exact-size
    per-expert groups (sum = T*K = 4096 columns, no capacity padding),
    then appends all T tokens once more as the "shared" group (2048 cols):
    one pairs matrix xp [D, 6144] = exactly 12 items of 512 columns.
  - The 8 shared experts (hidden FS=512 each, averaged) are algebraically
    one big FFN with hidden 8*512=4096; its per-core d_ff slice is exactly
    shared expert c.  So every core sees 9 uniform weight sets
    ([1024,512]/[1024,512]/[512,1024]): routed experts 0..7 (f-slice c)
    + shared expert c as weight-set 8.  alpha/NS (=1/16, exact) is folded
    into the shared w2 on the host.
  - Device: per item, ffn13 (w1/w3, 8 dt x 4 ft accumulation) -> silu*mul
    -> mm2 (w2) -> psum->sbuf bf16 copy -> DMA out.  Items may span expert
    boundaries; the per-(dt,ft) matmul is split at group boundaries
    (few ns extra per split).  mm2 is software-pipelined one item behind
    ffn13 so the silu/mul latency of the last ft never stalls the PE.
  - No gating on device: host multiplies routed outputs by (1-alpha)*prob
    during the combine (the standard MoE weighted-combine), so the PE does
    nothing but the 12*96 N=512 matmuls = the bf16 roofline.
  - Inputs stream on the Sync HWDGE queue in exact consumption order;
    outputs go on the Scalar HWDGE queue so their compute-gated waits can
    never head-of-line-block the input supply.
"""

import contextlib

import numpy as np
import ml_dtypes

import concourse.bacc as bacc
import concourse.tile as tile
import concourse.mybir as mybir
from concourse.bass_utils import run_bass_kernel_spmd

BF16 = ml_dtypes.bfloat16

B, S, D, F, E, NS, K = 2, 1024, 1024, 4096, 8, 8, 2
T = B * S
FS = F // NS            # shared expert hidden = 512 (= per-core routed slice)
FL = F // 8             # per-core f-slice of routed experts = 512
NW = E + 1              # 9 weight sets; wid 8 = shared
NCOL = T * K + T        # 6144 pair columns
IW = 512                # item width
NIT = NCOL // IW        # 12 items
ALPHA = 0.5
N_CORES = 8

# PE order of the 12 items: routed items r0..r7 occupy cols [512j, 512j+512),
# shared items s0..s3 are items 8..11 (cols 4096+).  Two shared items lead
# (tiny DMA footprint -> cheap prologue), the rest interleave for DMA slack.
PE_ORDER = [8, 9, 0, 1, 10, 2, 3, 11, 4, 5, 6, 7]

KCFG = {
    "warmup_mms": 56,    # garbage matmuls to warm the HAM clock gate and
                         # bridge the PE until the first input DMAs land
    "py_bufs": 3,
    "stream_last": 2,    # per-dt output DMA for the last N items
    "out_queue": "sync", # engine queue for output DMAs
}

_CACHE = {}         # sizes tuple -> compiled program
LAST_RESULT = None  # BassKernelResults of the most recent run (for profiling)


def _split_cost(sizes):
    """Extra PE ns caused by expert boundaries not landing on the 512 grid.

    Each of the 96 per-item (engine-step) rows issues one matmul per
    segment; a matmul costs max(25, N/2.4 + 2.5) ns, vs 215.8 for an
    unsplit 512 row.  Group sizes are all ~512+-30, so boundary offsets
    are small and the 60-cycle instruction floor dominates — the packing
    order of the groups controls how much of it we pay.
    """
    bounds = np.cumsum(sizes)
    tot = 0.0
    for j in range(8):
        lo, hi = 512 * j, 512 * j + 512
        cuts = [b for b in bounds[:-1] if lo < b < hi]
        pieces = np.diff([lo] + list(cuts) + [hi])
        tot += 96 * (sum(max(25.0, p / 2.4 + 2.5) for p in pieces)
                     - (512 / 2.4 + 2.5))
    return tot


def _best_perm(sizes):
    """Expert packing order minimizing the boundary split penalty
    (vectorized exhaustive search over all 8! orders)."""
    import itertools
    perms = np.array(list(itertools.permutations(range(E))), np.int64)
    s = np.asarray(sizes, np.int64)[perms]            # [P, 8]
    b = np.cumsum(s, axis=1)[:, :-1]                  # 7 boundaries
    d = (b % 512).astype(np.float64)
    pen = np.where(
        d > 0,
        np.maximum(25.0, d / 2.4 + 2.5)
        + np.maximum(25.0, (512 - d) / 2.4 + 2.5) - (512 / 2.4 + 2.5),
        0.0).sum(axis=1)
    return list(perms[int(np.argmin(pen))])


def _items_from_sizes(sizes):
    """Per-item segment lists [(wid, col_off_in_item, len), ...]."""
    bounds = np.cumsum([0] + list(sizes))
    items = []
    for j in range(8):                       # routed items
        lo, hi = IW * j, IW * j + IW
        segs = []
        for e in range(E):
            a, b = max(lo, int(bounds[e])), min(hi, int(bounds[e + 1]))
            if b > a:
                segs.append((e, a - lo, b - a))
        items.append(segs)
    for c in range(4):                       # shared items
        items.append([(E, 0, IW)])
    return items


def _build_program(sizes):
    bf = mybir.dt.bfloat16
    f32 = mybir.dt.float32
    Act = mybir.ActivationFunctionType

    items = _items_from_sizes(sizes)
    # first PE position at which each routed wid is used
    first_use = {}
    for pos, it in enumerate(PE_ORDER):
        for (wid, _, _) in items[it]:
            if wid != E and wid not in first_use:
                first_use[wid] = pos
    # weight-issue schedule: wid issued at block (first_use - 2)
    w_sched = {}
    for wid, fu in sorted(first_use.items(), key=lambda kv: kv[1]):
        w_sched.setdefault(max(0, fu - 2), []).append(wid)

    nc = bacc.Bacc("TRN2", target_bir_lowering=False, debug=False,
                   num_devices=N_CORES)

    xp = nc.dram_tensor("xp", [D, NCOL], bf, kind="ExternalInput").ap()
    w1 = nc.dram_tensor("w1", [NW, D, FL], bf, kind="ExternalInput").ap()
    w3 = nc.dram_tensor("w3", [NW, D, FL], bf, kind="ExternalInput").ap()
    w2 = nc.dram_tensor("w2", [NW, FL, D], bf, kind="ExternalInput").ap()
    y = nc.dram_tensor("y", [NIT, 128, 8, IW], bf, kind="ExternalOutput").ap()

    xp_r = xp.rearrange("(a p) t -> p a t", p=128)
    w1_r = [w1[i].rearrange("(a p) f -> p a f", p=128) for i in range(NW)]
    w3_r = [w3[i].rearrange("(a p) f -> p a f", p=128) for i in range(NW)]
    w2_r = [w2[i].rearrange("(a p) d -> p a d", p=128) for i in range(NW)]

    with tile.TileContext(nc) as tc:
        with contextlib.ExitStack() as ctx:
            const = ctx.enter_context(tc.tile_pool(name="const", bufs=1))
            wst = ctx.enter_context(tc.tile_pool(name="wst", bufs=4))
            acts = ctx.enter_context(tc.tile_pool(name="acts", bufs=4))
            hts = ctx.enter_context(tc.tile_pool(name="hts", bufs=2))
            spool = ctx.enter_context(tc.tile_pool(name="spool", bufs=2))
            outs = ctx.enter_context(tc.tile_pool(name="outs", bufs=2))
            psum = ctx.enter_context(
                tc.tile_pool(name="psum", bufs=2, space="PSUM"))
            psy = ctx.enter_context(
                tc.tile_pool(name="psy", bufs=KCFG["py_bufs"], space="PSUM"))
            psw = ctx.enter_context(
                tc.tile_pool(name="psw", bufs=1, space="PSUM"))

            state = {}
            out_dma = nc.sync.dma_start if KCFG["out_queue"] == "sync" \
                else nc.scalar.dma_start

            def load_w(wid):
                if wid == E:   # shared set: const pool, ft-split w1/w3
                    W1 = const.tile([128, 8, FL], bf, name="w1s")
                    W3 = const.tile([128, 8, FL], bf, name="w3s")
                    W2 = const.tile([128, 4, D], bf, name="w2s")
                else:
                    W1 = wst.tile([128, 8, FL], bf, tag="w1", name=f"w1_{wid}")
                    W3 = wst.tile([128, 8, FL], bf, tag="w3", name=f"w3_{wid}")
                    W2 = wst.tile([128, 4, D], bf, tag="w2", name=f"w2_{wid}")
                    nc.sync.dma_start(out=W1, in_=w1_r[wid])
                    nc.sync.dma_start(out=W3, in_=w3_r[wid])
                    nc.sync.dma_start(out=W2, in_=w2_r[wid])
                state[("W", wid)] = (W1, W3, W2)
                return W1, W3, W2

            def load_xp(pos):
                it = PE_ORDER[pos]
                o = it * IW
                XP = acts.tile([128, 8, IW], bf, tag="xp", name=f"xp{it}")
                nc.sync.dma_start(out=XP, in_=xp_r[:, :, o:o + IW])
                state[("XP", it)] = XP

            def ffn13(it):
                XP = state[("XP", it)]
                HT = hts.tile([128, 4, IW], bf, tag="ht", name=f"ht{it}")
                for ft in range(4):
                    fsl = slice(ft * 128, (ft + 1) * 128)
                    p1 = psum.tile([128, IW], f32, tag="p1", name=f"p1_{it}_{ft}")
                    p3 = psum.tile([128, IW], f32, tag="p3", name=f"p3_{it}_{ft}")
                    for wi, ps in ((0, p1), (1, p3)):
                        for (wid, o, ln) in items[it]:
                            W = state[("W", wid)][wi]
                            for dt in range(8):
                                nc.tensor.matmul(
                                    ps[:, o:o + ln], W[:, dt, fsl],
                                    XP[:, dt, o:o + ln],
                                    start=(dt == 0), stop=(dt == 7))
                    sa = spool.tile([128, IW], f32, tag="sa", name=f"sa{it}_{ft}")
                    nc.scalar.activation(sa, p1, Act.Silu)
                    nc.vector.tensor_mul(HT[:, ft, :], sa, p3)
                state[("HT", it)] = HT

            def mm2(it, stream_out=False):
                HT = state[("HT", it)]
                yo = outs.tile([128, 8, IW], bf, tag="yo", name=f"yo{it}")
                for dt in range(8):
                    dsl = slice(dt * 128, (dt + 1) * 128)
                    py = psy.tile([128, IW], f32, tag="py", name=f"py{it}_{dt}")
                    for (wid, o, ln) in items[it]:
                        W2t = state[("W", wid)][2]
                        for ft in range(4):
                            nc.tensor.matmul(
                                py[:, o:o + ln], W2t[:, ft, dsl],
                                HT[:, ft, o:o + ln],
                                start=(ft == 0), stop=(ft == 3))
                    nc.vector.tensor_copy(out=yo[:, dt, :], in_=py)
                    if stream_out:
                        out_dma(out=y[it, :, dt, :], in_=yo[:, dt, :])
                if not stream_out:
                    out_dma(out=y[it], in_=yo)

            # ---- HAM warmup: garbage matmuls fill the DMA-dead window --
            # The PE clock gate (HAM) needs ~3.4us of sustained activity to
            # un-throttle from 1.2 to 2.4 GHz.  Data DMAs cannot land before
            # ~9us, so burn that window on matmuls over uninitialized SBUF;
            # by the time real matmuls issue, the PE is already warm.
            nwu = KCFG["warmup_mms"]
            if nwu:
                wub = const.tile([128, 128], bf, name="wub")
                wup = psw.tile([128, 128], f32, tag="wu", name="wup")
                nc.vector.memset(wub, 0.0)
                for i in range(nwu):
                    nc.tensor.matmul(wup, wub, wub, start=True, stop=True)

            # ---- prologue: DMAs in exact consumption order -------------
            it0 = PE_ORDER[0]
            W1S, W3S, W2S = load_w(E)
            XP0 = acts.tile([128, 8, IW], bf, tag="xp", name=f"xp{it0}")
            state[("XP", it0)] = XP0
            o0 = it0 * IW
            nc.sync.dma_start(out=W1S[:, :, 0:128], in_=w1_r[E][:, :, 0:128])
            for q in range(4):
                nc.sync.dma_start(out=XP0[:, 2 * q:2 * q + 2, :],
                                  in_=xp_r[:, 2 * q:2 * q + 2, o0:o0 + IW])
                if q == 1:
                    nc.sync.dma_start(out=W3S[:, :, 0:128],
                                      in_=w3_r[E][:, :, 0:128])
            for ft in range(1, 4):
                fsl = slice(ft * 128, (ft + 1) * 128)
                nc.sync.dma_start(out=W1S[:, :, fsl], in_=w1_r[E][:, :, fsl])
                nc.sync.dma_start(out=W3S[:, :, fsl], in_=w3_r[E][:, :, fsl])
            nc.sync.dma_start(out=W2S, in_=w2_r[E])
            load_xp(1)
            for wid in w_sched.get(0, []):
                load_w(wid)
            load_xp(2)

            # ---- main loop: mm2 lags ffn13 by one item -----------------
            nstream = KCFG["stream_last"]
            for pos in range(NIT):
                if pos >= 1:
                    for wid in w_sched.get(pos, []):
                        load_w(wid)
                    if pos + 2 < NIT:
                        load_xp(pos + 2)
                ffn13(PE_ORDER[pos])
                if pos >= 1:
                    mm2(PE_ORDER[pos - 1],
                        stream_out=(pos - 1 >= NIT - nstream))
            mm2(PE_ORDER[NIT - 1], stream_out=True)

    nc.compile()
    return nc


def kernel(hidden_states, gate_W, w1_e, w3_e, w2_e, w1_s, w3_s, w2_s):
    global LAST_RESULT
    x = np.ascontiguousarray(np.asarray(hidden_states, np.float32).reshape(T, D))

    # ---- host routing (sharding decision) + combine coefficients ----
    gate_W = np.asarray(gate_W, np.float32)
    logits = x @ gate_W.T                       # [T, E]
    m = logits.max(axis=1, keepdims=True)
    p = np.exp(logits - m)
    probs = p / p.sum(axis=1, keepdims=True)
    order = np.argsort(-probs, axis=1, kind="stable")[:, :K]   # [T, K]

    idx = [np.where((order == e).any(axis=1))[0] for e in range(E)]
    nsz = [len(te) for te in idx]
    assert sum(nsz) == T * K
    perm = _best_perm(nsz)                      # packing order of the groups
    sizes = tuple(nsz[e] for e in perm)

    # ---- build device inputs ----------------------------------------
    xT = np.ascontiguousarray(x.T)              # [D, T] fp32
    xf_bf = xT.astype(BF16)                     # [D, T]
    xp_bf = np.empty((D, NCOL), dtype=BF16)
    off = 0
    for e in perm:
        n = len(idx[e])
        xp_bf[:, off:off + n] = xf_bf[:, idx[e]]
        off += n
    xp_bf[:, T * K:] = xf_bf                    # shared group: all tokens

    w1_e = np.asarray(w1_e, np.float32)
    w3_e = np.asarray(w3_e, np.float32)
    w2_e = np.asarray(w2_e, np.float32)
    w1_s = np.asarray(w1_s, np.float32)
    w3_s = np.asarray(w3_s, np.float32)
    # fold alpha/NS (an exact power of two) into the shared down-proj
    w2_s = np.asarray(w2_s, np.float32) * (ALPHA / NS)

    nc = _CACHE.get(sizes)
    if nc is None:
        nc = _CACHE[sizes] = _build_program(sizes)

    in_maps = []
    for c in range(N_CORES):
        fsl = slice(c * FL, (c + 1) * FL)
        w1c = np.concatenate(
            [np.ascontiguousarray(w1_e[perm][:, :, fsl]), w1_s[c:c + 1]],
            axis=0)
        w3c = np.concatenate(
            [np.ascontiguousarray(w3_e[perm][:, :, fsl]), w3_s[c:c + 1]],
            axis=0)
        w2c = np.concatenate(
            [np.ascontiguousarray(w2_e[perm][:, fsl, :]), w2_s[c:c + 1]],
            axis=0)
        in_maps.append({
            "xp": xp_bf,
            "w1": w1c.astype(BF16),
            "w3": w3c.astype(BF16),
            "w2": w2c.astype(BF16),
        })

    res = run_bass_kernel_spmd(nc, in_maps, list(range(N_CORES)))
    LAST_RESULT = res

    # ---- host combine (unshard + weighted MoE combine) --------------
    yfull = np.zeros((NIT, 128, 8, IW), np.float32)
    for c in range(N_CORES):
        yfull += res.results[c]["y"].astype(np.float32)
    # [it, p, a, t] -> [a*128+p, it*512+t] = [D, NCOL]
    yfull = np.ascontiguousarray(yfull.transpose(2, 1, 0, 3)).reshape(D, NCOL)

    outT = yfull[:, T * K:].copy()              # shared part (scales folded)
    off = 0
    for e in perm:
        te = idx[e]
        coef = ((1.0 - ALPHA) * probs[te, e]).astype(np.float32)
        outT[:, te] += yfull[:, off:off + len(te)] * coef[None, :]
        off += len(te)

    return np.ascontiguousarray(outT.T).reshape(B, S, D).astype(np.float32)
